# revision 39
# baseline (speedup 1.0000x reference)
"""Trainium2 Bass kernel for nn_Bottleneck (sparse 3x3 gather-GEMM bottleneck block).

Strategy (8 NeuronCores, zero cross-core communication):
  - The neighbor graph (19% occupancy on a 1024^2 grid, Moore stencil) has tiny
    connected components (max ~54). Host bin-packs whole components into 200
    bins of 1024 tokens (8 cores x 25 pairs), so every neighbor reference is
    PAIR-local and the whole block fuses into one per-pair pipeline.
  - Per pair: conv1 (channel-major, LN-centering folded into weights) ->
    DMA-transpose to a token-major per-pair gather table -> LN1+GELU applied
    in-place on the table (rstd as the Activation's per-partition scale, a
    -30 bias masks padding slots) -> GPSIMD dma_gather (2 subtiles) -> conv2
    gather-GEMM -> LN2 (variance via ones-matmul, rstd broadcast via GPSIMD
    partition_broadcast) -> LN3 variance BEFORE conv3 via Cholesky factor L
    (L L^T = W3' W3'^T / 256), rstd3 folded into conv3's rhs -> conv3 ->
    residual + GELU -> store.  No DRAM roundtrips, no broadcast DMAs, x is
    read once (bf16) and reused for the residual.
"""

import sys

sys.path.insert(0, "/opt/trn_rl_repo")

import numpy as np

import concourse.bass as bass
import concourse.tile as tile
from concourse import bacc as bacc_mod
from concourse import mybir
from concourse.bass_utils import run_bass_kernel_spmd

# Problem constants (hardcoded per contract).
N = 200000
C_IN = 256
C_MID = 64
EPS = 1e-6
NCORES = 8

PAIR = 1024                # tokens per pair (one fused pipeline stage)
SUB = 512                  # PSUM free-dim limit for f32
NPAIR = 25                 # pairs per core
T = NPAIR * PAIR           # 25600 padded tokens per core
NBINS = NCORES * NPAIR     # 200 global bins
GSUB = 9 * SUB             # gather indices per subtile
NRANK = PAIR // 128 + 1    # 8 data ranks + 1 sentinel rank per pair table
SENT = PAIR                # pair-local sentinel id -> zeroed rank

f32 = mybir.dt.float32
f32r = mybir.dt.float32r
i32 = mybir.dt.int32
bf16 = mybir.dt.bfloat16
i16 = mybir.dt.int16
AF = mybir.ActivationFunctionType
OP = mybir.AluOpType

_NC_CACHE = {}
DEBUG_DUMPS = False


def _ap(src_ap, dims):
    """Manual access pattern with explicit [step, count] dims over src."""
    return bass.AP(tensor=src_ap.tensor, offset=src_ap.offset, ap=dims)

NDBG = 4


def build_nc(simple_ln1=True):
    key = ("nc", simple_ln1, DEBUG_DUMPS)
    if key in _NC_CACHE:
        return _NC_CACHE[key]
    nc = bacc_mod.Bacc(None, target_bir_lowering=False, debug=False)

    if DEBUG_DUMPS:
        dbg_tab = nc.declare_dram_parameter(
            "dbg_tab", [NDBG, 128, NRANK * 128], bf16, isOutput=True)
        dbg_gath = nc.declare_dram_parameter(
            "dbg_gath", [NDBG, 128, 2, GSUB], bf16, isOutput=True)

    x_t = nc.declare_dram_parameter("x_t", [2, 128, T], bf16, isOutput=False)
    w1 = nc.declare_dram_parameter("w1", [128, 2, C_MID], bf16, isOutput=False)
    w2 = nc.declare_dram_parameter("w2", [C_MID, 9, C_MID], bf16, isOutput=False)
    w3 = nc.declare_dram_parameter("w3", [128, C_IN], bf16, isOutput=False)
    lmat = nc.declare_dram_parameter("lmat", [128, 128], bf16, isOutput=False)
    es2 = nc.declare_dram_parameter("es2", [128, 2], bf16, isOutput=False)  # col c: half-c indicator
    ebc = nc.declare_dram_parameter("ebc", [2, 128], f32, isOutput=False)  # row r: half-r out indicator
    g1b = nc.declare_dram_parameter("g1b", [128, C_MID], bf16, isOutput=False)
    b1b = nc.declare_dram_parameter("b1b", [128, C_MID], f32, isOutput=False)
    g2r = nc.declare_dram_parameter("g2r", [128, 1], f32, isOutput=False)
    b2r = nc.declare_dram_parameter("b2r", [128, 1], f32, isOutput=False)
    g3r = nc.declare_dram_parameter("g3r", [128, 2], f32, isOutput=False)
    b3r = nc.declare_dram_parameter("b3r", [128, 2], f32, isOutput=False)
    maskb = nc.declare_dram_parameter("maskb", [128, 8 * NPAIR], f32, isOutput=False)
    idx = nc.declare_dram_parameter("idx", [NPAIR, 128, 2 * GSUB // 16], i16, isOutput=False)
    y_t = nc.declare_dram_parameter("y_t", [2, 128, T], f32, isOutput=True)

    from contextlib import ExitStack

    with ExitStack() as ctx:
        tc = ctx.enter_context(tile.TileContext(nc))
        consts = ctx.enter_context(tc.tile_pool(name="consts", bufs=1))
        xp = ctx.enter_context(tc.tile_pool(name="xin", bufs=6))
        ip = ctx.enter_context(tc.tile_pool(name="idxp", bufs=6))
        asp = ctx.enter_context(tc.tile_pool(name="a1s", bufs=4))
        tp = ctx.enter_context(tc.tile_pool(name="table", bufs=2))
        vp = ctx.enter_context(tc.tile_pool(name="var1", bufs=8))
        scp = ctx.enter_context(tc.tile_pool(name="scr", bufs=1))
        gp = ctx.enter_context(tc.tile_pool(name="gath", bufs=4))
        sqp = ctx.enter_context(tc.tile_pool(name="sq", bufs=4))
        rp2 = ctx.enter_context(tc.tile_pool(name="rstd", bufs=4))
        rbp = ctx.enter_context(tc.tile_pool(name="rbc", bufs=6))
        h2p = ctx.enter_context(tc.tile_pool(name="h2n", bufs=3))
        h2cp = ctx.enter_context(tc.tile_pool(name="h2c", bufs=3))
        yp = ctx.enter_context(tc.tile_pool(name="yout", bufs=4))
        psA = ctx.enter_context(tc.tile_pool(name="psA", bufs=2, space="PSUM"))
        psH = ctx.enter_context(tc.tile_pool(name="psH", bufs=2, space="PSUM"))
        psS = ctx.enter_context(tc.tile_pool(name="psS", bufs=2, space="PSUM"))
        psC = ctx.enter_context(tc.tile_pool(name="psC", bufs=2, space="PSUM"))

        # ---- constants into SBUF ----
        w1_sb = consts.tile([128, 2, C_MID], bf16)
        nc.sync.dma_start(out=w1_sb[:], in_=w1[:])
        w2_sb = consts.tile([C_MID, 9, C_MID], bf16)
        nc.sync.dma_start(out=w2_sb[:], in_=w2[:])
        w3_sb = consts.tile([128, C_IN], bf16)
        nc.sync.dma_start(out=w3_sb[:], in_=w3[:])
        l2_sb = consts.tile([128, 128], bf16)
        nc.sync.dma_start(out=l2_sb[:], in_=lmat[:])
        es2_sb = consts.tile([128, 2], bf16)
        nc.sync.dma_start(out=es2_sb[:], in_=es2[:])
        ebc_sb = consts.tile([2, 128], f32r)
        nc.sync.dma_start(out=ebc_sb[:], in_=ebc[:].bitcast(f32r))
        g2_sb = consts.tile([128, 1], f32, tag="g2")
        nc.sync.dma_start(out=g2_sb[:], in_=g2r[:])
        b2_sb = consts.tile([128, 1], f32, tag="b2")
        nc.sync.dma_start(out=b2_sb[:], in_=b2r[:])
        g3_sb = consts.tile([128, 2], f32, tag="g3")
        nc.sync.dma_start(out=g3_sb[:], in_=g3r[:])
        b3_sb = consts.tile([128, 2], f32, tag="b3")
        nc.sync.dma_start(out=b3_sb[:], in_=b3r[:])
        mask_sb = consts.tile([128, 8 * NPAIR], f32)
        nc.sync.dma_start(out=mask_sb[:], in_=maskb[:])
        if not simple_ln1:
            g1b_sb = consts.tile([128, C_MID], bf16)
            nc.sync.dma_start(out=g1b_sb[:], in_=g1b[:])
            b1b_sb = consts.tile([128, C_MID], f32)
            nc.sync.dma_start(out=b1b_sb[:], in_=b1b[:])
        eps_sb = consts.tile([128, 1], f32, tag="eps")
        nc.vector.memset(eps_sb[:], EPS)

        nidx_reg = nc.gpsimd.to_reg(2 * GSUB)

        # Two table buffers, fully zeroed once: the per-stripe pad bytes and
        # the sentinel rank stay zero forever; transposes rewrite data halves.
        NTAB = 4
        tables = []
        for i in range(NTAB):
            t_ = tp.tile([128, NRANK * 128], bf16, tag=f"table{i}")
            nc.vector.memset(t_[:], 0.0)
            tables.append(t_)

        def emit_loads(p):
            x_sb = xp.tile([128, 2, PAIR], bf16, tag="xin")
            nc.sync.dma_start(
                out=x_sb[:],
                in_=x_t[:, :, p * PAIR:(p + 1) * PAIR].rearrange("c p f -> p c f"),
            )
            idx_sb = ip.tile([128, 2 * GSUB // 16], i16, tag="idx")
            nc.sync.dma_start(out=idx_sb[:], in_=idx[p])
            return x_sb, idx_sb

        st = {}

        def emit_front2(grp):
            fd = {}
            for p in grp:
                x_sb, idx_sb = loads.pop(p)
                fd[p] = dict(x_sb=x_sb, idx_sb=idx_sb)
            # ---- conv1 (channel-major) ----
            for p in grp:
                a1 = psA.tile([128, SUB], f32, tag="A1", name=f"a1_{p}")
                for s in range(2):
                    for c in range(2):
                        nc.tensor.matmul(
                            out=a1[64 * s:64 * s + 64, :],
                            lhsT=w1_sb[:, c, :],
                            rhs=fd[p]["x_sb"][:, c, SUB * s:SUB * (s + 1)],
                            start=(c == 0),
                            stop=(c == 1),
                            tile_position=(0, 64 * s),
                        )
                fd[p]["a1"] = a1
            for p in grp:
                a1s = asp.tile([128, SUB], bf16, tag="a1s", name=f"a1s_{p}")
                nc.scalar.copy(out=a1s[:], in_=fd[p]["a1"][:])
                fd[p]["a1s"] = a1s
            # ---- token-major table via DMA transposes (split SP/Act) ----
            for p in grp:
                table = tables[p % NTAB]
                blocks = []
                for b in range(8):
                    s, q = b // 4, b % 4
                    blk = table[:, 128 * b:128 * b + 64]
                    blocks.append(blk)
                    nc.sync.dma_start_transpose(
                        out=blk,
                        in_=fd[p]["a1s"][64 * s:64 * s + 64,
                                         128 * q:128 * (q + 1)],
                    )
                fd[p]["table"] = table
                fd[p]["blocks"] = blocks
            # ---- LN1 variance + Newton rsqrt (DVE only) ----
            for p in grp:
                var1 = vp.tile([128, 8], f32, tag="var1", name=f"v1_{p}")
                scr = scp.tile([128, C_MID], f32, tag="scr")
                for b in range(8):
                    nc.vector.scalar_tensor_tensor(
                        out=scr[:], in0=fd[p]["blocks"][b], scalar=1.0,
                        in1=fd[p]["blocks"][b],
                        op0=OP.mult, op1=OP.mult,
                        accum_out=var1[:, b:b + 1],
                    )
                fd[p]["var1"] = var1
            for p in grp:
                var1 = fd[p]["var1"]
                nc.vector.tensor_scalar(
                    out=var1[:], in0=var1[:], scalar1=1.0 / C_MID, scalar2=EPS,
                    op0=OP.mult, op1=OP.add,
                )
                rstd1 = vp.tile([128, 8], f32, tag="rstd1", name=f"r1_{p}")
                t3 = vp.tile([128, 8], f32, tag="nrt3", name=f"t3_{p}")
                ri = rstd1[:].bitcast(i32)
                nc.vector.tensor_scalar(out=ri, in0=var1[:].bitcast(i32),
                                        scalar1=1, scalar2=None,
                                        op0=OP.logical_shift_right)
                nc.vector.tensor_scalar(out=ri, in0=ri, scalar1=0, scalar2=None,
                                        op0=OP.bitwise_not)
                nc.vector.tensor_scalar(out=ri, in0=ri, scalar1=0x5f3759e0,
                                        scalar2=None, op0=OP.add)
                for _ in range(2):
                    nc.vector.scalar_tensor_tensor(
                        out=t3[:], in0=rstd1[:], scalar=1.0, in1=rstd1[:],
                        op0=OP.mult, op1=OP.mult)
                    nc.vector.scalar_tensor_tensor(
                        out=t3[:], in0=t3[:], scalar=-0.5, in1=var1[:],
                        op0=OP.mult, op1=OP.mult)
                    nc.vector.scalar_tensor_tensor(
                        out=rstd1[:], in0=t3[:], scalar=1.5, in1=rstd1[:],
                        op0=OP.add, op1=OP.mult)
                fd[p]["rstd1"] = rstd1
            # ---- LN1 scale+mask+GELU in place on the table ----
            for p in grp:
                for b in range(8):
                    blk = fd[p]["blocks"][b]
                    if simple_ln1:
                        nc.scalar.activation(
                            out=blk, in_=blk, func=AF.Gelu,
                            scale=fd[p]["rstd1"][:, b:b + 1],
                            bias=mask_sb[:, 8 * p + b:8 * p + b + 1],
                        )
                    else:
                        nc.vector.scalar_tensor_tensor(
                            out=blk, in0=blk, scalar=fd[p]["rstd1"][:, b:b + 1],
                            in1=g1b_sb[:], op0=OP.mult, op1=OP.mult,
                        )
                        nc.vector.scalar_tensor_tensor(
                            out=blk, in0=blk, scalar=1.0,
                            in1=b1b_sb[:], op0=OP.mult, op1=OP.add,
                        )
                        nc.scalar.activation(
                            out=blk, in_=blk, func=AF.Gelu,
                            bias=mask_sb[:, 8 * p + b:8 * p + b + 1],
                        )
            for p in grp:
                st[p] = dict(x_sb=fd[p]["x_sb"], idx_sb=fd[p]["idx_sb"],
                             a1s=fd[p]["a1s"], table=fd[p]["table"])
                emit_gather(p)

        def emit_gather(p):
            d = st[p]
            table, idx_sb, a1s = d["table"], d["idx_sb"], d["a1s"]
            if DEBUG_DUMPS and p < NDBG:
                nc.sync.dma_start(out=dbg_tab[p], in_=table[:])
            gath = gp.tile([128, 1, 2 * GSUB], bf16, tag="gath")
            nc.gpsimd.dma_gather(
                out_ap=gath[:],
                in_ap=table[:],
                idxs_ap=idx_sb[:],
                num_idxs=2 * GSUB,
                num_idxs_reg=nidx_reg,
                elem_size=128,
                transpose=True,
                sbuf_tokens_per_rank=128,
                sbuf_free_dim_per_rank=256,
                sbuf_free_dim_pad_per_rank=0,
                sbuf_byte_offset=0,
                single_packet=False,
            )
            # WAR guards: SWDGE gather source reads are released before the
            # data phase completes, so couple the next pair's writers of
            # table/idx/a1s to gather completion via tiny reads of gath
            # (gath data landing implies all gather source reads finished,
            # and transitively that the transposes finished reading a1s).
            nc.vector.tensor_scalar_mul(
                out=table[0:1, 0:1024:128], in0=gath[0:1, 0, 0:8], scalar1=1.0,
            )
            nc.vector.tensor_scalar_mul(
                out=idx_sb[0:1, 0:1].bitcast(bf16),
                in0=gath[0:1, 0, 0:1], scalar1=1.0,
            )
            nc.vector.tensor_scalar_mul(
                out=a1s[0:1, 0:8], in0=gath[0:1, 0, 0:8], scalar1=1.0,
            )
            d["gath"] = gath

        def emit_back2(grp):
            for p in grp:
                d = st[p]
                if DEBUG_DUMPS and p < NDBG:
                    nc.sync.dma_start(out=dbg_gath[p], in_=d["gath"])
                # ---- conv2 gather-GEMM ----
                gath = d["gath"]
                h2 = psH.tile([128, SUB], f32, tag="H2", name=f"H2_{p}")
                d["h2"] = h2
                for s in range(2):
                    for k in range(9):
                        nc.tensor.matmul(
                            out=h2[64 * s:64 * s + 64, :],
                            lhsT=w2_sb[:, k, :],
                            rhs=gath[0:64, 0,
                                     s * GSUB + SUB * k:s * GSUB + SUB * (k + 1)],
                            start=(k == 0),
                            stop=(k == 8),
                            tile_position=(0, 64 * s),
                        )
            # ---- LN2 ----
            for p in grp:
                d = st[p]
                sq2 = sqp.tile([128, SUB], bf16, tag="sq", name=f"sq2_{p}")
                nc.scalar.activation(out=sq2[:], in_=d["h2"][:], func=AF.Square)
                d["sq2"] = sq2
            for p in grp:
                d = st[p]
                grid2 = psS.tile([128, SUB], f32, tag="grid", name=f"g2_{p}")
                for s in range(2):
                    nc.tensor.matmul(
                        out=grid2[64 * s:64 * s + 1, :],
                        lhsT=es2_sb[:, s:s + 1], rhs=d["sq2"][:],
                        start=True, stop=True, tile_position=(0, 64 * s),
                    )
                d["grid2"] = grid2
            for p in grp:
                d = st[p]
                rstd2 = rp2.tile([1, 2, SUB], f32, tag="rstd", name=f"r2_{p}")
                for s in range(2):
                    nc.scalar.activation(
                        out=rstd2[0:1, s, :], in_=d["grid2"][64 * s:64 * s + 1, :],
                        func=AF.Sqrt, bias=eps_sb[0:1, :], scale=1.0 / C_MID,
                    )
                d["rstd2"] = rstd2
            for p in grp:
                nc.vector.reciprocal(out=st[p]["rstd2"][:], in_=st[p]["rstd2"][:])
            for p in grp:
                d = st[p]
                rb2 = [rbp.tile([128, SUB], f32, tag="rb", name=f"rb2_{p}_{s_}")
                       for s_ in range(2)]
                for s in range(2):
                    nc.gpsimd.partition_broadcast(rb2[s][:], d["rstd2"][0:1, s, :])
                d["rb2"] = rb2
            for p in grp:
                d = st[p]
                h2n = h2p.tile([128, SUB], bf16, tag="h2n", name=f"h2n_{p}")
                for s in range(2):
                    h = slice(64 * s, 64 * s + 64)
                    nc.vector.scalar_tensor_tensor(
                        out=h2n[h, :], in0=d["h2"][h, :], scalar=g2_sb[h, :],
                        in1=d["rb2"][s][h, :], op0=OP.mult, op1=OP.mult,
                    )
                d["h2n"] = h2n
            for p in grp:
                nc.scalar.activation(out=st[p]["h2n"][:], in_=st[p]["h2n"][:],
                                     func=AF.Gelu, bias=b2_sb[:])

            # ---- LN3 variance via Cholesky factor (before conv3) ----
            for p in grp:
                d = st[p]
                u = psS.tile([128, SUB], f32, tag="grid", name=f"u_{p}")
                nc.tensor.matmul(out=u[:], lhsT=l2_sb[:], rhs=d["h2n"][:],
                                 start=True, stop=True)
                d["u"] = u
            for p in grp:
                d = st[p]
                squ = sqp.tile([128, SUB], bf16, tag="sq", name=f"squ_{p}")
                nc.scalar.activation(out=squ[:], in_=d["u"][:], func=AF.Square)
                d["squ"] = squ
            for p in grp:
                d = st[p]
                grid3 = psS.tile([128, SUB], f32, tag="grid", name=f"g3_{p}")
                for s in range(2):
                    nc.tensor.matmul(
                        out=grid3[64 * s:64 * s + 1, :],
                        lhsT=es2_sb[:, s:s + 1], rhs=d["squ"][:],
                        start=True, stop=True, tile_position=(0, 64 * s),
                    )
                d["grid3"] = grid3
            for p in grp:
                d = st[p]
                rstd3 = rp2.tile([1, 2, SUB], f32, tag="rstd", name=f"r3_{p}")
                for s in range(2):
                    nc.scalar.activation(
                        out=rstd3[0:1, s, :],
                        in_=d["grid3"][64 * s:64 * s + 1, :],
                        func=AF.Sqrt, bias=eps_sb[0:1, :], scale=1.0,
                    )
                d["rstd3"] = rstd3
            for p in grp:
                nc.vector.reciprocal(out=st[p]["rstd3"][:], in_=st[p]["rstd3"][:])
            for p in grp:
                d = st[p]
                rb3 = [rbp.tile([128, SUB], f32, tag="rb", name=f"rb3_{p}_{s_}")
                       for s_ in range(2)]
                for s in range(2):
                    nc.gpsimd.partition_broadcast(rb3[s][:], d["rstd3"][0:1, s, :])
                d["rb3"] = rb3
            for p in grp:
                d = st[p]
                h2c = h2cp.tile([128, SUB], bf16, tag="h2c", name=f"h2c_{p}")
                for s in range(2):
                    h = slice(64 * s, 64 * s + 64)
                    nc.vector.scalar_tensor_tensor(
                        out=h2c[h, :], in0=d["h2n"][h, :], scalar=1.0,
                        in1=d["rb3"][s][h, :], op0=OP.mult, op1=OP.mult,
                    )
                d["h2c"] = h2c

            # ---- conv3 + residual + GELU + store ----
            for p in grp:
                d = st[p]
                y_sb = yp.tile([128, 2, PAIR], f32, tag="y", name=f"y_{p}")
                for s in range(2):
                    for c in range(2):
                        a3 = psC.tile([128, SUB], f32, tag="A3", name=f"a3_{p}")
                        nc.tensor.matmul(
                            out=a3[:],
                            lhsT=w3_sb[64 * s:64 * s + 64, 128 * c:128 * (c + 1)],
                            rhs=d["h2c"][64 * s:64 * s + 64, :],
                            start=True, stop=True,
                            tile_position=(64 * s, 0),
                        )
                        nc.vector.scalar_tensor_tensor(
                            out=y_sb[:, c, SUB * s:SUB * (s + 1)],
                            in0=a3[:], scalar=g3_sb[:, c:c + 1],
                            in1=d["x_sb"][:, c, SUB * s:SUB * (s + 1)],
                            op0=OP.mult, op1=OP.add,
                        )
                d["y_sb"] = y_sb
            for p in grp:
                for c in range(2):
                    nc.scalar.activation(
                        out=st[p]["y_sb"][:, c, :], in_=st[p]["y_sb"][:, c, :],
                        func=AF.Gelu, bias=b3_sb[:, c:c + 1],
                    )
            for p in grp:
                d = st.pop(p)
                nc.gpsimd.dma_start(
                    out=y_t[:, :, p * PAIR:(p + 1) * PAIR].rearrange(
                        "c p f -> p c f"),
                    in_=d["y_sb"][:],
                )

        # Software-pipelined emission over 2-pair groups: loads 2 groups
        # ahead, fronts (conv1..LN1+gather) 1 group ahead, then the grouped
        # back-end (2 pairs per stage, halving Act function-set reloads).
        groups = [list(range(g, min(g + 2, NPAIR))) for g in range(0, NPAIR, 2)]
        loads = {}
        for pr in groups[0] + (groups[1] if len(groups) > 1 else []):
            loads[pr] = emit_loads(pr)
        emit_front2(groups[0])
        for gi, grp in enumerate(groups):
            if gi + 2 < len(groups):
                for pr in groups[gi + 2]:
                    loads[pr] = emit_loads(pr)
            if gi + 1 < len(groups):
                emit_front2(groups[gi + 1])
            emit_back2(grp)

    nc.compile()
    _NC_CACHE[key] = nc
    return nc


# ======================= host-side sharding =======================

def _components(nbr):
    """Connected-component labels via vectorized min-label propagation."""
    lab = np.arange(N, dtype=np.int64)
    ks = [k for k in range(9) if k != 4]
    valid = [(nbr[k] < N) for k in ks]
    nbrs = [nbr[k].astype(np.int64) for k in ks]
    for _ in range(200):
        new = lab.copy()
        for k in range(len(ks)):
            v = valid[k]
            cand = lab[nbrs[k][v]]
            np.minimum.at(new, np.nonzero(v)[0], cand)
        new = np.minimum(new, new[new])
        if np.array_equal(new, lab):
            break
        lab = new
    while True:
        new = lab[lab]
        if np.array_equal(new, lab):
            break
        lab = new
    return lab


def _binpack(nbr):
    """First-fit-decreasing of whole components into NBINS bins of PAIR slots.

    Returns bins: list of per-bin np arrays of global token ids.
    """
    lab = _components(nbr)
    comp_ids, comp_inv, comp_sizes = np.unique(
        lab, return_inverse=True, return_counts=True
    )
    order = np.argsort(comp_sizes)[::-1]
    resid = np.full(NBINS, PAIR, dtype=np.int64)
    comp_bin = np.empty(len(comp_ids), dtype=np.int64)
    for ci in order:
        sz = comp_sizes[ci]
        b = int(np.argmax(resid >= sz))
        assert resid[b] >= sz, f"component of size {sz} does not fit"
        comp_bin[ci] = b
        resid[b] -= sz
    # tokens grouped by bin, components contiguous within a bin
    point_bin = comp_bin[comp_inv]
    order_tok = np.lexsort((comp_inv, point_bin))
    bins = []
    bounds = np.searchsorted(point_bin[order_tok], np.arange(NBINS + 1))
    for b in range(NBINS):
        bins.append(order_tok[bounds[b]:bounds[b + 1]])
    return bins


def _prep_core(x_bf, nbr, core_bins):
    """Build per-core x_t (bf16), idx blob, maskb, and slot->global id map."""
    import ml_dtypes
    ids = np.full(T, -1, dtype=np.int64)          # slot -> global token id
    for p, bin_ids in enumerate(core_bins):
        ids[p * PAIR:p * PAIR + len(bin_ids)] = bin_ids
    real = ids >= 0
    glob2slot = np.full(N + 1, -1, dtype=np.int64)
    glob2slot[ids[real]] = np.nonzero(real)[0]

    xl = np.zeros((T, C_IN), dtype=ml_dtypes.bfloat16)
    xl[real] = x_bf[ids[real]]
    x_t = np.ascontiguousarray(xl.T.reshape(2, 128, T))

    # pair-local neighbor slots; sentinel PAIR for missing/out-of-pair
    nbl = np.full((9, T), SENT, dtype=np.int64)
    src = ids[real]
    nb = nbr[:, src].astype(np.int64)             # [9, nreal] global ids (N = missing)
    nb_slot = np.where(nb < N, glob2slot[np.clip(nb, 0, N - 1)], -1)
    tgt_slot = np.nonzero(real)[0]
    same_pair = (nb_slot >= 0) & ((nb_slot // PAIR) == (tgt_slot[None, :] // PAIR))
    assert bool(np.all(same_pair | (nb_slot < 0))), "neighbor escaped its pair"
    nbl[:, tgt_slot] = np.where(same_pair, nb_slot % PAIR, SENT)

    # idx blob: per pair one merged list [s-major, k-major, token order],
    # 16-wrapped, x8 replicated -> [NPAIR, 128, 576]
    flat = np.empty((NPAIR, 2, GSUB), dtype=np.int16)
    for k in range(9):
        seg = nbl[k].reshape(NPAIR, 2, SUB).astype(np.int16)
        flat[:, :, k * SUB:(k + 1) * SUB] = seg
    flat = flat.reshape(NPAIR, 2 * GSUB)
    wrapped = flat.reshape(NPAIR, 2 * GSUB // 16, 16).transpose(0, 2, 1)
    idx_blob = np.ascontiguousarray(np.tile(wrapped, (1, 8, 1)))

    # per-block gelu bias mask: 0 for real slots, -30 for padding
    mask = np.where(real, 0.0, -30.0).astype(np.float32)
    maskb = np.ascontiguousarray(
        mask.reshape(8 * NPAIR, 128).T               # block-major -> [128, 200]
    )
    return x_t, idx_blob, maskb, ids


def _prep_weights(W1, W2, W3, g1, b1, g2, b2, g3, b3):
    import ml_dtypes
    W1 = np.asarray(W1, np.float64)
    W2 = np.asarray(W2, np.float64)
    W3 = np.asarray(W3, np.float64)

    def center(w):
        return w - w.mean(axis=-1, keepdims=True)

    W1c = center(W1)                 # [256, 64]
    W2c = center(W2)                 # [9, 64, 64]
    W3c = center(W3)                 # [64, 256]
    w1 = np.ascontiguousarray(
        W1c.reshape(2, 128, C_MID).transpose(1, 0, 2)
        .astype(np.float32).astype(ml_dtypes.bfloat16)
    )
    w2 = np.ascontiguousarray(
        W2c.transpose(1, 0, 2).astype(np.float32).astype(ml_dtypes.bfloat16)
    )
    w3 = np.ascontiguousarray(
        np.tile(W3c.astype(np.float32), (2, 1)).astype(ml_dtypes.bfloat16)
    )
    M3 = (W3c @ W3c.T) / C_IN
    L = np.linalg.cholesky(M3 + 1e-12 * np.eye(C_MID))
    l2 = np.zeros((128, 128), np.float64)
    l2[0:64, 0:64] = L
    l2[64:128, 64:128] = L
    lmat = np.ascontiguousarray(l2.astype(np.float32).astype(ml_dtypes.bfloat16))
    es2 = np.zeros((128, 2), np.float32)
    es2[0:64, 0] = 1.0
    es2[64:128, 1] = 1.0
    es2 = es2.astype(ml_dtypes.bfloat16)
    ebc = np.zeros((2, 128), np.float32)
    ebc[0, 0:64] = 1.0
    ebc[1, 64:128] = 1.0

    def rep2(v):
        return np.ascontiguousarray(
            np.tile(np.asarray(v, np.float32).reshape(C_MID), 2).reshape(128, 1)
        )

    g2r, b2r = rep2(g2), rep2(b2)
    g3r = np.ascontiguousarray(np.asarray(g3, np.float32).reshape(2, 128).T)
    b3r = np.ascontiguousarray(np.asarray(b3, np.float32).reshape(2, 128).T)
    g1 = np.asarray(g1, np.float32).reshape(C_MID)
    b1 = np.asarray(b1, np.float32).reshape(C_MID)
    simple_ln1 = bool(np.all(g1 == 1.0) and np.all(b1 == 0.0))
    g1b = np.ascontiguousarray(
        np.tile(g1, (128, 1)).astype(ml_dtypes.bfloat16)
    )
    b1b = np.ascontiguousarray(np.tile(b1, (128, 1)))
    return dict(
        w1=w1, w2=w2, w3=w3, lmat=lmat, es2=es2, ebc=ebc, g1b=g1b, b1b=b1b,
        g2r=g2r, b2r=b2r, g3r=g3r, b3r=b3r,
    ), simple_ln1


def kernel(x, W1, W2, W3, g1, b1, g2, b2, g3, b3, neighbor_idx):
    import ml_dtypes
    x_bf = np.asarray(x, np.float32).astype(ml_dtypes.bfloat16)
    nbr = np.asarray(neighbor_idx)
    wmap, simple_ln1 = _prep_weights(W1, W2, W3, g1, b1, g2, b2, g3, b3)
    bins = _binpack(nbr)
    in_maps = []
    metas = []
    for c in range(NCORES):
        x_t, idx_blob, maskb, ids = _prep_core(
            x_bf, nbr, bins[c * NPAIR:(c + 1) * NPAIR]
        )
        metas.append(ids)
        in_maps.append(dict(x_t=x_t, idx=idx_blob, maskb=maskb, **wmap))
    nc = build_nc(simple_ln1)
    res = run_bass_kernel_spmd(nc, in_maps, core_ids=list(range(NCORES)))
    y = np.empty((N, C_IN), dtype=np.float32)
    for c in range(NCORES):
        yt = res.results[c]["y_t"]                # [2, 128, T]
        ids = metas[c]
        real = ids >= 0
        yl = yt.reshape(C_IN, T).T                # [T, 256]
        y[ids[real]] = yl[real]
    return y
